# revision 14
# baseline (speedup 1.0000x reference)
"""CrystalGraphConvNet Bass/Tile kernel for TRN2 (8-core data-parallel).

Device algorithm (per core, 2 crystals, BJ=192 bj-rows, R=18432 (bj,k)-rows):
  - gated = conv(total) computed as ONE augmented bf16 matmul per row-block:
      lhsT [128, 64] = [W3 ; 0 ; A'^T_block ; B^T_block], rhs_pack [128, cols] =
      [nbrT ; 0 ; ones-diag ; adj-diag]  -> raw gated in PSUM, partition-packed
      (filt(H0)/filt(H1) stacked to use all 128 lanes downstream).
  - bn1 stats computed analytically (no pass over gated): host supplies
    layer-independent nbr/adj reductions (Gram term, nbrsum@W3, S1/S1a);
    device adds the fea-dependent linear/quadratic terms via tiny matmul
    contractions; per layer one 8-core AllGather of [128,6] partial sums
    (cheaper floor than AllReduce) + local 8-slot tree reduce.
  - sigmoid via ACT Sigmoid table (bn1 folded into per-partition scale/bias);
    core-half drained from PSUM on DVE with the bn1 affine folded in; softplus
    per chunk as Exp+Ln on ACT (natural_log_exp table, 2 table loads/layer).
  - h = sig*sp on DVE; k-sum via contiguous-halves add tree (bf16 2x).
  - bn2: free-dim reduce + AllGather [128,2] + local reduce; fea update
    via Softplus.
"""

import numpy as np
import ml_dtypes

import concourse.bass as bass
import concourse.mybir as mybir
from concourse import tile

F32 = mybir.dt.float32
BF16 = mybir.dt.bfloat16
I32 = mybir.dt.int32
AF = mybir.ActivationFunctionType
OP = mybir.AluOpType

EPS = 1e-5
N0, N, ORIG, F, K, H, NC = 16, 96, 92, 64, 41, 128, 3
NCORES, BPC = 8, 2
BJ = BPC * N            # 192
R = BJ * N              # 18432
G32 = 32
NBLK = BJ // G32        # 6
HALF = R // 2           # 9216
NTOT = float(N0 * N * N)
NTOT2 = float(N0 * N)
NGRP = 18               # main groups per layer, 512 paired-cols each
GW = 512
SPCH = 3                # softplus/mul/tree chunks
CHW = HALF // SPCH      # 2304 = 24 bj * 96


def bf16(x):
    return np.ascontiguousarray(np.asarray(x, np.float32).astype(ml_dtypes.bfloat16))


INPUT_SPECS = [
    ("rhs_pack", (128, R), BF16),
    ("atomT", (ORIG + 1, BJ), F32),
    ("emb", (ORIG + 1, F), F32),
    ("w3", (K, NC * 128), BF16),
    ("wab", (F + 1, NC * 256), F32),
    ("s1s", (G32, NC * NBLK * 256), BF16),
    ("aux", (F, 8), BF16),
    ("gvec", (128, 12), F32),
    ("gvec2", (F, 6), F32),
    ("fcW", (F, H), F32),
    ("fcb", (H, 1), F32),
    ("outW", (H, 1), F32),
    ("outb", (1, 1), F32),
]


def host_prep(inputs):
    """Build the 8 per-core input maps from the full problem inputs."""
    atom_fea = np.asarray(inputs["atom_fea"], np.float32)
    nbr_fea = np.asarray(inputs["nbr_fea"], np.float32)
    adj = np.asarray(inputs["adj"])
    conv_W = np.asarray(inputs["conv_W"], np.float64)
    conv_b = np.asarray(inputs["conv_b"], np.float64)

    emb_ext = np.concatenate(
        [np.asarray(inputs["emb_W"], np.float32),
         np.asarray(inputs["emb_b"], np.float32)[None]], 0)
    w3_all = np.concatenate([bf16(conv_W[l, 2 * F:]) for l in range(NC)], 1)
    wab_all = np.concatenate(
        [np.concatenate(
            [np.concatenate([conv_W[l, :F], conv_b[l][None]], 0),
             np.concatenate([conv_W[l, F:2 * F], np.zeros((1, 2 * F))], 0)], 1)
         for l in range(NC)], 1).astype(np.float32)
    fcW = np.asarray(inputs["fc_W"], np.float32)
    # negated: consumed as the Sigmoid nbias inside _softplus (see kernel)
    fcb = -np.asarray(inputs["fc_b"], np.float32).reshape(H, 1)
    outW = np.asarray(inputs["out_W"], np.float32).reshape(H, 1)
    outb = np.asarray(inputs["out_b"], np.float32).reshape(1, 1)
    bn1_g = np.asarray(inputs["bn1_g"], np.float32)
    bn1_b = np.asarray(inputs["bn1_b"], np.float32)
    bn2_g = np.asarray(inputs["bn2_g"], np.float32)
    bn2_b = np.asarray(inputs["bn2_b"], np.float32)

    colbj = np.arange(R) // N
    gidx = colbj % G32

    per_core, nbrsum_g, gram_g = [], 0.0, 0.0
    for c in range(NCORES):
        sl = slice(c * BPC, (c + 1) * BPC)
        nbr = nbr_fea[sl].reshape(R, K).astype(np.float64)
        adjf = adj[sl].reshape(R).astype(np.float64)
        deg = adjf.reshape(BJ, N).sum(1)
        rhs = np.zeros((128, R), np.float32)
        rhs[0:K] = nbr.T
        rhs[64 + gidx, np.arange(R)] = 1.0
        rhs[96 + gidx, np.arange(R)] = adjf
        nbrj = nbr.reshape(BJ, N, K).sum(1)
        nbrja = (nbr.reshape(BJ, N, K) * adjf.reshape(BJ, N, 1)).sum(1)
        s1s = np.empty((G32, NC * NBLK * 256), np.float64)
        for l in range(NC):
            W3 = conv_W[l, 2 * F:]
            S1T, S1aT = nbrj @ W3, nbrja @ W3
            for b in range(NBLK):
                blk = np.concatenate(
                    [S1T[b * G32:(b + 1) * G32], S1aT[b * G32:(b + 1) * G32]], 1)
                s1s[:, (l * NBLK + b) * 256:(l * NBLK + b + 1) * 256] = blk
        aux = np.zeros((F, 8), np.float64)
        for b in range(NBLK):
            aux[0:32, b] = deg[b * G32:(b + 1) * G32]
            aux[32:64, b] = deg[b * G32:(b + 1) * G32]
        aux[0:64, 6] = 1.0
        atomT = np.concatenate(
            [atom_fea[sl].reshape(BJ, ORIG).T, np.ones((1, BJ))], 0).astype(np.float32)
        nbrsum_g = nbrsum_g + nbr.sum(0)
        gram_g = gram_g + nbr.T @ nbr
        per_core.append(dict(rhs=bf16(rhs), atomT=atomT, s1s=bf16(s1s), aux=bf16(aux)))

    # Core-half bn1 params and bn2 gain are negated host-side: the kernel
    # computes softplus(z) as -Ln(Sigmoid(-z)), so the core affine must
    # produce -z, and the resulting negated `summed` is fixed up in bn2 by
    # the negated gain (bias formula is sign-invariant).
    gvec = np.zeros((128, 12), np.float32)
    for l in range(NC):
        W3 = conv_W[l, 2 * F:]
        gvec[:, l] = nbrsum_g @ W3
        gvec[:, 3 + l] = np.einsum("fc,fg,gc->c", W3, gram_g, W3)
        gvec[:, 6 + l] = bn1_g[l]
        gvec[F:128, 6 + l] *= -1.0
        gvec[:, 9 + l] = bn1_b[l]
        gvec[F:128, 9 + l] *= -1.0
    gvec2 = np.zeros((F, 6), np.float32)
    for l in range(NC):
        gvec2[:, l] = -bn2_g[l]
        gvec2[:, 3 + l] = bn2_b[l]

    in_maps = []
    for c in range(NCORES):
        pc = per_core[c]
        in_maps.append({
            "rhs_pack": pc["rhs"], "atomT": pc["atomT"], "emb": emb_ext,
            "w3": w3_all, "wab": wab_all, "s1s": pc["s1s"], "aux": pc["aux"],
            "gvec": gvec, "gvec2": gvec2, "fcW": fcW, "fcb": fcb,
            "outW": outW, "outb": outb,
        })
    return in_maps


def _softplus(nc, pool, out, in_, tag, nbias=0.0, nscale=-1.0):
    """out = softplus(x) via -Ln(Sigmoid(-x)); pass nscale=-scale, nbias=-bias.

    Using only {Sigmoid, Ln} keeps every ACT in the kernel inside two
    tables (sigmoid_and_others / natural_log) instead of thrashing the
    Exp<->Ln table pair on every softplus."""
    p, fd = out.shape[0], int(np.prod(out.shape[1:]))
    e = pool.tile([p, fd], F32, tag=tag + "_e")
    nc.scalar.activation(e[:], in_, AF.Sigmoid, bias=nbias, scale=nscale)
    nc.scalar.activation(out, e[:], AF.Ln)
    nc.vector.tensor_scalar(out, out, -1.0, None, OP.mult)


def _rsqrt(nc, pool, out, v, p):
    """out = 1/sqrt(v), [p,1] f32, via magic-init + 3 Newton iterations."""
    yb = pool.tile([p, 1], I32, tag="rs_i")
    nc.vector.tensor_scalar(yb[:], v.bitcast(I32), 1, None, OP.logical_shift_right)
    nc.vector.tensor_scalar(yb[:], yb[:], -1, 0x5F3759DF, OP.mult, OP.add)
    y = yb.bitcast(F32)
    t = pool.tile([p, 1], F32, tag="rs_t")
    for _ in range(2):
        nc.vector.tensor_tensor(t[:], y[:], y[:], OP.mult)
        nc.vector.tensor_tensor(t[:], t[:], v[:], OP.mult)
        nc.vector.tensor_scalar(t[:], t[:], -0.5, 1.5, OP.mult, OP.add)
        nc.vector.tensor_tensor(y[:], y[:], t[:], OP.mult)
    nc.vector.tensor_copy(out, y[:])


def trace_body(nc, d, out_ap):
    """d: dict name -> DRAM AP (inputs); out_ap: [1,2] f32 DRAM output."""
    # Collective bounce buffers: outputs must be addr_space="Shared" on HW.
    # AllGather (floor ~2x cheaper than AllReduce) + local 8-slot reduce.
    ar1b = [(nc.dram_tensor(f"ar1i_{l}", [128, 6], F32).ap(),
             nc.dram_tensor(f"ar1o_{l}", [NCORES, 128, 6], F32,
                            addr_space="Shared").ap())
            for l in range(NC)]
    ar2b = [(nc.dram_tensor(f"ar2i_{l}", [128, 2], F32).ap(),
             nc.dram_tensor(f"ar2o_{l}", [NCORES, 128, 2], F32,
                            addr_space="Shared").ap())
            for l in range(NC)]
    dmy_in = nc.dram_tensor("dmy_i", [1, 1], F32).ap()
    dmy_out = nc.dram_tensor("dmy_o", [NCORES, 1, 1], F32,
                             addr_space="Shared").ap()
    with tile.TileContext(nc) as tc:
        with (
            tc.tile_pool(name="big", bufs=1) as big,
            tc.tile_pool(name="cst", bufs=1) as cst,
            tc.tile_pool(name="ph0", bufs=2) as ph0,
            tc.tile_pool(name="gate", bufs=2) as gate,
            tc.tile_pool(name="sm", bufs=2) as sm,
            tc.tile_pool(name="ps_ab", bufs=1, space="PSUM") as ps_ab_p,
            tc.tile_pool(name="ps_st", bufs=1, space="PSUM") as ps_st_p,
            tc.tile_pool(name="ps_f", bufs=3, space="PSUM") as ps_f_p,
            tc.tile_pool(name="ps_c", bufs=2, space="PSUM") as ps_c_p,
            tc.tile_pool(name="ps_m", bufs=1, space="PSUM") as ps_m_p,
        ):
            # Dummy first collective: absorbs cross-core launch skew and CC
            # warmup under the input DMAs + phase0 instead of delaying AR1.
            nc.gpsimd.collective_compute(
                "AllGather", OP.bypass, replica_groups=[list(range(NCORES))],
                ins=[dmy_in], outs=[dmy_out])
            # ---- load constants (rhs_pack last: only needed by the main
            # matmuls ~40us in; issuing it first would stall the small loads
            # behind a 14us DMA and delay phase0 + the first AllGather) ----
            rhs_pack = big.tile([128, R], BF16)
            atomT = cst.tile([ORIG + 1, BJ], F32)
            nc.sync.dma_start(atomT[:], d["atomT"])
            emb = cst.tile([ORIG + 1, F], F32)
            nc.sync.dma_start(emb[:], d["emb"])
            w3 = cst.tile([K, NC * 128], BF16)
            nc.sync.dma_start(w3[:], d["w3"])
            wab = cst.tile([F + 1, NC * 256], F32)
            nc.sync.dma_start(wab[:], d["wab"])
            s1s = cst.tile([G32, NC * NBLK * 256], BF16)
            nc.sync.dma_start(s1s[:], d["s1s"])
            aux = cst.tile([F, 8], BF16)
            nc.sync.dma_start(aux[:], d["aux"])
            gvec = cst.tile([128, 12], F32)
            nc.sync.dma_start(gvec[:], d["gvec"])
            gvec2 = cst.tile([F, 6], F32)
            nc.sync.dma_start(gvec2[:], d["gvec2"])
            fcW = cst.tile([F, H], F32)
            nc.sync.dma_start(fcW[:], d["fcW"])
            fcb = cst.tile([H, 1], F32)
            nc.sync.dma_start(fcb[:], d["fcb"])
            outW = cst.tile([H, 1], F32)
            nc.sync.dma_start(outW[:], d["outW"])
            outb = cst.tile([1, 1], F32)
            nc.sync.dma_start(outb[:], d["outb"])
            nc.sync.dma_start(rhs_pack[:], d["rhs_pack"])

            # ---- embedding: fea_ext [65, 192] = [(atom@embW+b)^T ; ones] ----
            ps_e = ps_m_p.tile([F, BJ], F32, tag="pm")
            nc.tensor.matmul(ps_e[:], emb[:], atomT[:], start=True, stop=True)
            fea_ext = ph0.tile([F + 1, BJ], F32, tag="fea")
            nc.vector.tensor_copy(fea_ext[0:F, :], ps_e[:])
            nc.vector.memset(fea_ext[F:F + 1, :], 1.0)

            for l in range(NC):
                # ================= phase 0: A'/B, lhsT blocks, bn1 stat terms
                lhs_all = ph0.tile([128, NBLK * 2 * F], BF16, tag="lhs")
                nc.vector.memset(lhs_all[:], 0.0)  # rows 41:64 must stay zero
                ps_st = ps_st_p.tile([128, 8], F32, tag="st")
                # ab2 [32, 256] per block = [A'^T | B^T] at base partition 0
                # (TT inputs must share base partition when both in SBUF);
                # kept for all 6 blocks so lhsT assembly can run AFTER the
                # AR1 trigger, inside the collective's latency shadow.
                ab2_all = ph0.tile([G32, NBLK * 256], BF16, tag="ab_sb")
                for b in range(NBLK):
                    o = b * 256
                    ps_ab = ps_ab_p.tile([128, 128], F32, tag="ab")
                    fsl = fea_ext[:, b * G32:(b + 1) * G32]
                    nc.tensor.matmul(ps_ab[64:96, :], fsl,
                                     wab[:, l * 256:l * 256 + 128],
                                     start=True, stop=True, tile_position=(0, 64))
                    nc.tensor.matmul(ps_ab[96:128, :], fsl,
                                     wab[:, l * 256 + 128:l * 256 + 256],
                                     start=True, stop=True, tile_position=(0, 96))
                    nc.vector.tensor_copy(ab2_all[:, o:o + 128], ps_ab[64:96, :])
                    nc.vector.tensor_copy(ab2_all[:, o + 128:o + 256],
                                          ps_ab[96:128, :])
                    sq = ph0.tile([G32, 256], BF16, tag="sq")
                    nc.vector.tensor_tensor(sq[:], ab2_all[:, o:o + 256],
                                            ab2_all[:, o:o + 256], OP.mult)
                    prod = ph0.tile([G32, 128], BF16, tag="prod")
                    nc.vector.tensor_tensor(prod[:], ab2_all[:, o:o + 128],
                                            ab2_all[:, o + 128:o + 256], OP.mult)
                    crs = ph0.tile([G32, 256], BF16, tag="crs")
                    nc.vector.tensor_tensor(
                        crs[:], ab2_all[:, o:o + 256],
                        s1s[:, (l * NBLK + b) * 256:(l * NBLK + b + 1) * 256],
                        OP.mult)
                    # stat contractions -> ps_st cols 0..5
                    st, sp_ = (b == 0), (b == NBLK - 1)
                    nc.tensor.matmul(ps_st[:, 0:1], ab2_all[:, o:o + 128],
                                     aux[0:32, 6:7],
                                     start=st, stop=sp_, skip_group_check=True)
                    nc.tensor.matmul(ps_st[:, 1:2], ab2_all[:, o + 128:o + 256],
                                     aux[0:32, b:b + 1],
                                     start=st, stop=sp_, skip_group_check=True)
                    nc.tensor.matmul(ps_st[:, 2:3], sq[:, 0:128], aux[0:32, 6:7],
                                     start=st, stop=sp_, skip_group_check=True)
                    nc.tensor.matmul(ps_st[:, 3:4], sq[:, 128:256],
                                     aux[0:32, b:b + 1],
                                     start=st, stop=sp_, skip_group_check=True)
                    nc.tensor.matmul(ps_st[:, 4:5], prod[:], aux[0:32, b:b + 1],
                                     start=st, stop=sp_, skip_group_check=True)
                    nc.tensor.matmul(ps_st[:, 5:6], crs[:, 0:128], aux[0:32, 6:7],
                                     start=st, stop=False, skip_group_check=True)
                    nc.tensor.matmul(ps_st[:, 5:6], crs[:, 128:256], aux[0:32, 6:7],
                                     start=False, stop=sp_, skip_group_check=True)

                # ---- AR1: AllGather partials + local 8-slot tree reduce ----
                ar_sb = sm.tile([128, 6], F32, tag="ar1s")
                nc.vector.tensor_copy(ar_sb[:], ps_st[:, 0:6])
                ar_in, ar_out = ar1b[l]
                nc.sync.dma_start(ar_in, ar_sb[:])
                nc.gpsimd.collective_compute(
                    "AllGather", OP.bypass, replica_groups=[list(range(NCORES))],
                    ins=[ar_in], outs=[ar_out])

                # lhsT assembly (W3 + A'/B rows), hidden under AR1 latency
                for b in range(NBLK):
                    o = b * 256
                    fcol, ccol = 2 * b * F, (2 * b + 1) * F
                    nc.vector.tensor_copy(lhs_all[0:K, fcol:fcol + F],
                                          w3[:, l * 128:l * 128 + F])
                    nc.vector.tensor_copy(lhs_all[0:K, ccol:ccol + F],
                                          w3[:, l * 128 + F:(l + 1) * 128])
                    nc.vector.tensor_copy(lhs_all[64:96, fcol:fcol + F],
                                          ab2_all[:, o:o + F])
                    nc.vector.tensor_copy(lhs_all[64:96, ccol:ccol + F],
                                          ab2_all[:, o + F:o + 128])
                    nc.vector.tensor_copy(lhs_all[96:128, fcol:fcol + F],
                                          ab2_all[:, o + 128:o + 128 + F])
                    nc.vector.tensor_copy(lhs_all[96:128, ccol:ccol + F],
                                          ab2_all[:, o + 128 + F:o + 256])

                argg = sm.tile([128, NCORES * 6], F32, tag="ar1gg")
                nc.sync.dma_start(
                    argg[:], ar_out.rearrange("r p c -> p r c"))
                arg4 = sm.tile([128, 24], F32, tag="ar1g4")
                nc.vector.tensor_tensor(arg4[:], argg[:, 0:24], argg[:, 24:48],
                                        OP.add)
                arg2r = sm.tile([128, 12], F32, tag="ar1g2")
                nc.vector.tensor_tensor(arg2r[:], arg4[:, 0:12], arg4[:, 12:24],
                                        OP.add)
                arg = sm.tile([128, 6], F32, tag="ar1g")
                nc.vector.tensor_tensor(arg[:], arg2r[:, 0:6], arg2r[:, 6:12],
                                        OP.add)

                # ---- main matmuls (raw gated) + sig + drain ----
                sig_buf = gate.tile([128, HALF], BF16, tag="sig")
                spin_buf = gate.tile([128, HALF], BF16, tag="spin")

                # bn1 finalize (tiny, f32)  -- runs when arg ready
                sg = sm.tile([128, 1], F32, tag="sg")
                nc.vector.tensor_scalar(sg[:], arg[:, 0:1], float(N), None, OP.mult)
                nc.vector.tensor_tensor(sg[:], sg[:], arg[:, 1:2], OP.add)
                nc.vector.tensor_tensor(sg[:], sg[:], gvec[:, l:l + 1], OP.add)
                sg2 = sm.tile([128, 1], F32, tag="sg2")
                nc.vector.tensor_scalar(sg2[:], arg[:, 2:3], float(N), None, OP.mult)
                nc.vector.tensor_tensor(sg2[:], sg2[:], arg[:, 3:4], OP.add)
                t45 = sm.tile([128, 1], F32, tag="t45")
                nc.vector.tensor_tensor(t45[:], arg[:, 4:5], arg[:, 5:6], OP.add)
                nc.vector.tensor_scalar(t45[:], t45[:], 2.0, None, OP.mult)
                nc.vector.tensor_tensor(sg2[:], sg2[:], t45[:], OP.add)
                nc.vector.tensor_tensor(sg2[:], sg2[:], gvec[:, 3 + l:4 + l], OP.add)
                mean = sm.tile([128, 1], F32, tag="mean")
                nc.vector.tensor_scalar(mean[:], sg[:], 1.0 / NTOT, None, OP.mult)
                var = sm.tile([128, 1], F32, tag="var")
                nc.vector.tensor_tensor(var[:], mean[:], mean[:], OP.mult)
                ex2 = sm.tile([128, 1], F32, tag="ex2")
                nc.vector.tensor_scalar(ex2[:], sg2[:], 1.0 / NTOT, None, OP.mult)
                nc.vector.tensor_tensor(var[:], ex2[:], var[:], OP.subtract)
                nc.vector.tensor_scalar(var[:], var[:], EPS, None, OP.add)
                inv = sm.tile([128, 1], F32, tag="inv")
                _rsqrt(nc, sm, inv[:], var, 128)
                scl = sm.tile([128, 1], F32, tag="scl")
                nc.vector.tensor_tensor(scl[:], gvec[:, 6 + l:7 + l], inv[:], OP.mult)
                bia = sm.tile([128, 1], F32, tag="bia")
                nc.vector.tensor_tensor(bia[:], mean[:], scl[:], OP.mult)
                nc.vector.tensor_tensor(bia[:], gvec[:, 9 + l:10 + l], bia[:],
                                        OP.subtract)
                # filt-half bn1 affine, duplicated to both partition halves
                sigscl = sm.tile([128, 1], F32, tag="sigscl")
                nc.vector.tensor_copy(sigscl[0:F, :], scl[0:F, :])
                nc.vector.tensor_copy(sigscl[F:128, :], scl[0:F, :])
                sigbia = sm.tile([128, 1], F32, tag="sigbia")
                nc.vector.tensor_copy(sigbia[0:F, :], bia[0:F, :])
                nc.vector.tensor_copy(sigbia[F:128, :], bia[0:F, :])
                spscl = sm.tile([128, 1], F32, tag="spscl")
                nc.vector.tensor_copy(spscl[0:F, :], scl[F:128, :])
                nc.vector.tensor_copy(spscl[F:128, :], scl[F:128, :])
                spbia = sm.tile([128, 1], F32, tag="spbia")
                nc.vector.tensor_copy(spbia[0:F, :], bia[F:128, :])
                nc.vector.tensor_copy(spbia[F:128, :], bia[F:128, :])

                for g in range(NGRP):
                    bp, j = g // 6, g % 6
                    c0 = bp * 3072 + j * GW
                    c1 = HALF + c0
                    gc = c0
                    ps_f = ps_f_p.tile([128, GW], F32, tag="psf")
                    ps_c = ps_c_p.tile([128, GW], F32, tag="psc")
                    fa = lhs_all[:, 2 * bp * F:(2 * bp + 1) * F]
                    fb = lhs_all[:, 2 * (bp + 3) * F:(2 * (bp + 3) + 1) * F]
                    ca = lhs_all[:, (2 * bp + 1) * F:(2 * bp + 2) * F]
                    cb = lhs_all[:, (2 * (bp + 3) + 1) * F:(2 * (bp + 3) + 2) * F]
                    nc.tensor.matmul(ps_f[0:F, :], fa, rhs_pack[:, c0:c0 + GW],
                                     start=True, stop=True)
                    nc.tensor.matmul(ps_f[F:128, :], fb, rhs_pack[:, c1:c1 + GW],
                                     start=True, stop=True)
                    nc.tensor.matmul(ps_c[0:F, :], ca, rhs_pack[:, c0:c0 + GW],
                                     start=True, stop=True)
                    nc.tensor.matmul(ps_c[F:128, :], cb, rhs_pack[:, c1:c1 + GW],
                                     start=True, stop=True)
                    nc.scalar.activation(sig_buf[:, gc:gc + GW], ps_f[:],
                                         AF.Sigmoid, bias=sigbia[:],
                                         scale=sigscl[:])
                    nc.vector.tensor_scalar(spin_buf[:, gc:gc + GW], ps_c[:],
                                            spscl[:], spbia[:],
                                            OP.mult, OP.add)

                # ---- softplus + mul + k-reduce tree, chunked pipeline ----
                summed = sm.tile([128, 2 * BPC * G32 * 3 // 4], F32, tag="summed")
                # summed [128, 96]: p<64 -> (c, bj 0:96), p>=64 -> (c, bj 96:192)
                for ch in range(SPCH):
                    co = ch * CHW
                    nbj = CHW // N  # 24
                    # spin holds -(bn1 core affine); Sigmoid+Ln gives
                    # -softplus, absorbed by the host-negated bn2 gain.
                    s_t = gate.tile([128, CHW], BF16, tag="e")
                    nc.scalar.activation(s_t[:], spin_buf[:, co:co + CHW],
                                         AF.Sigmoid)
                    sp_t = gate.tile([128, CHW], BF16, tag="sp")
                    nc.scalar.activation(sp_t[:], s_t[:], AF.Ln)
                    h_t = gate.tile([128, CHW], BF16, tag="h")
                    nc.vector.tensor_tensor(h_t[:], sig_buf[:, co:co + CHW],
                                            sp_t[:], OP.mult)
                    # tree: 96 -> 48 -> 24 -> 12 -> 6 -> 3 -> (2 adds)
                    w = N
                    cur = h_t
                    while w > 3:
                        nw = w // 2
                        nxt = gate.tile([128, nbj * nw], BF16, tag=f"tr{w}")
                        va = cur[:].rearrange("p (b k) -> p b k", k=w)
                        nc.vector.tensor_tensor(
                            nxt[:].rearrange("p (b k) -> p b k", k=nw),
                            va[:, :, 0:nw], va[:, :, nw:2 * nw], OP.add)
                        cur, w = nxt, nw
                    va = cur[:].rearrange("p (b k) -> p b k", k=3)
                    s01 = gate.tile([128, nbj], F32, tag="s01")
                    nc.vector.tensor_tensor(
                        s01[:].rearrange("p (b k) -> p b k", k=1),
                        va[:, :, 0:1], va[:, :, 1:2], OP.add)
                    nc.vector.tensor_tensor(
                        summed[:, ch * nbj:(ch + 1) * nbj].rearrange(
                            "p (b k) -> p b k", k=1),
                        s01[:].rearrange("p (b k) -> p b k", k=1),
                        va[:, :, 2:3], OP.add)

                # ---- bn2 ----
                NB2 = 2 * BPC * G32 * 3 // 4  # 96
                ar2_sb = sm.tile([128, 2], F32, tag="ar2s")
                nc.vector.tensor_reduce(ar2_sb[:, 0:1], summed[:],
                                        axis=mybir.AxisListType.X, op=OP.add)
                ssq = sm.tile([128, NB2], F32, tag="ssq")
                nc.vector.tensor_tensor(ssq[:], summed[:], summed[:], OP.mult)
                nc.vector.tensor_reduce(ar2_sb[:, 1:2], ssq[:],
                                        axis=mybir.AxisListType.X, op=OP.add)
                ar2_in, ar2_out = ar2b[l]
                nc.sync.dma_start(ar2_in, ar2_sb[:])
                nc.gpsimd.collective_compute(
                    "AllGather", OP.bypass, replica_groups=[list(range(NCORES))],
                    ins=[ar2_in], outs=[ar2_out])
                ar2gg = sm.tile([128, NCORES * 2], F32, tag="ar2gg")
                nc.sync.dma_start(
                    ar2gg[:], ar2_out.rearrange("r p c -> p r c"))
                ar2g4 = sm.tile([128, 8], F32, tag="ar2g4")
                nc.vector.tensor_tensor(ar2g4[:], ar2gg[:, 0:8], ar2gg[:, 8:16],
                                        OP.add)
                ar2g2 = sm.tile([128, 4], F32, tag="ar2g2")
                nc.vector.tensor_tensor(ar2g2[:], ar2g4[:, 0:4], ar2g4[:, 4:8],
                                        OP.add)
                arg2 = sm.tile([128, 2], F32, tag="ar2g")
                nc.vector.tensor_tensor(arg2[:], ar2g2[:, 0:2], ar2g2[:, 2:4],
                                        OP.add)
                arg2c = sm.tile([F, 2], F32, tag="arg2c")
                nc.vector.tensor_copy(arg2c[:], arg2[F:128, :])
                g2 = sm.tile([F, 2], F32, tag="g2")
                nc.vector.tensor_tensor(g2[:], arg2[0:F, :], arg2c[:], OP.add)
                m2 = sm.tile([F, 1], F32, tag="m2")
                nc.vector.tensor_scalar(m2[:], g2[:, 0:1], 1.0 / NTOT2, None, OP.mult)
                v2 = sm.tile([F, 1], F32, tag="v2")
                nc.vector.tensor_tensor(v2[:], m2[:], m2[:], OP.mult)
                e2 = sm.tile([F, 1], F32, tag="e2")
                nc.vector.tensor_scalar(e2[:], g2[:, 1:2], 1.0 / NTOT2, None, OP.mult)
                nc.vector.tensor_tensor(v2[:], e2[:], v2[:], OP.subtract)
                nc.vector.tensor_scalar(v2[:], v2[:], EPS, None, OP.add)
                i2 = sm.tile([F, 1], F32, tag="i2")
                _rsqrt(nc, sm, i2[:], v2, F)
                s2 = sm.tile([F, 1], F32, tag="s2")
                nc.vector.tensor_tensor(s2[:], gvec2[:, l:l + 1], i2[:], OP.mult)
                b2 = sm.tile([F, 1], F32, tag="b2")
                nc.vector.tensor_tensor(b2[:], m2[:], s2[:], OP.mult)
                nc.vector.tensor_tensor(b2[:], gvec2[:, 3 + l:4 + l], b2[:],
                                        OP.subtract)
                s2d = sm.tile([128, 1], F32, tag="s2d")
                nc.vector.tensor_copy(s2d[0:F, :], s2[:])
                nc.vector.tensor_copy(s2d[F:128, :], s2[:])
                b2d = sm.tile([128, 1], F32, tag="b2d")
                nc.vector.tensor_copy(b2d[0:F, :], b2[:])
                nc.vector.tensor_copy(b2d[F:128, :], b2[:])
                sn = sm.tile([128, NB2], F32, tag="sn")
                nc.vector.tensor_scalar(sn[:], summed[:], s2d[:], b2d[:],
                                        OP.mult, OP.add)
                snc = sm.tile([F, NB2], F32, tag="snc")
                nc.vector.tensor_copy(snc[:], sn[F:128, :])
                tmp = sm.tile([F, BJ], F32, tag="tmpf")
                nc.vector.tensor_tensor(tmp[:, 0:N], sn[0:F, :],
                                        fea_ext[0:F, 0:N], OP.add)
                nc.vector.tensor_tensor(tmp[:, N:BJ], snc[:],
                                        fea_ext[0:F, N:BJ], OP.add)
                fea_new = ph0.tile([F + 1, BJ], F32, tag="fea")
                _softplus(nc, sm, fea_new[0:F, :], tmp[:], "feasp")
                nc.vector.memset(fea_new[F:F + 1, :], 1.0)
                fea_ext = fea_new

            # ---- head ----
            crys = sm.tile([F, BPC], F32, tag="crys")
            nc.vector.tensor_reduce(
                crys[:], fea_ext[0:F, :].rearrange("p (a b) -> p a b", b=N),
                axis=mybir.AxisListType.X, op=OP.add)
            nc.vector.tensor_scalar(crys[:], crys[:], 1.0 / N, None, OP.mult)
            crys2 = sm.tile([F, BPC], F32, tag="crys2")
            _softplus(nc, sm, crys2[:], crys[:], "hd1")
            ps_h = ps_m_p.tile([H, BPC], F32, tag="pm")
            nc.tensor.matmul(ps_h[:], fcW[:], crys2[:], start=True, stop=True)
            sph = sm.tile([H, BPC], F32, tag="sph")
            _softplus(nc, sm, sph[:], ps_h[:], "hd2", nbias=fcb[:])
            ps_o = ps_m_p.tile([1, BPC], F32, tag="pm")
            nc.tensor.matmul(ps_o[:], outW[:], sph[:], start=True, stop=True)
            res = sm.tile([1, BPC], F32, tag="res")
            nc.vector.tensor_scalar(res[:], ps_o[:], outb[0:1, 0:1], None, OP.add)
            nc.sync.dma_start(out_ap, res[:])  # out dram is [1, BPC]
    return nc


# ======================================================================
# Self-contained runner: shard -> compile (cached) -> run SPMD -> gather
# ======================================================================
_COMPILED = {}


def _build_nc():
    import concourse.bacc as bacc
    nc = bacc.Bacc("TRN2", target_bir_lowering=False, debug=False,
                   num_devices=NCORES)
    d = {}
    for name, shape, dt in INPUT_SPECS:
        d[name] = nc.dram_tensor(name, list(shape), dt, kind="ExternalInput").ap()
    out_ap = nc.dram_tensor("out", [1, BPC], F32, kind="ExternalOutput").ap()
    trace_body(nc, d, out_ap)
    nc.compile()
    return nc


def kernel(**inputs):
    from concourse.bass_utils import run_bass_kernel_spmd
    in_maps = host_prep(inputs)
    if "nc" not in _COMPILED:
        _COMPILED["nc"] = _build_nc()
    nc = _COMPILED["nc"]
    res = run_bass_kernel_spmd(nc, in_maps, core_ids=list(range(NCORES)))
    out = np.concatenate([np.asarray(r["out"], np.float32).reshape(BPC)
                          for r in res.results])
    return out.reshape(N0, 1)



# revision 25
# speedup vs baseline: 1.0581x; 1.0581x over previous
"""CrystalGraphConvNet Bass/Tile kernel for TRN2 (8-core data-parallel).

Device algorithm (per core, 2 crystals, BJ=192 bj-rows, R=18432 (bj,k)-rows):
  - gated = conv(total) computed as ONE augmented bf16 matmul per row-block:
      lhsT [128, 64] = [W3 ; 0 ; A'^T_block ; B^T_block], rhs_pack [128, cols] =
      [nbrT ; 0 ; ones-diag ; adj-diag]  -> raw gated in PSUM, partition-packed
      (filt(H0)/filt(H1) stacked to use all 128 lanes downstream).
  - bn1 stats computed analytically (no pass over gated): host supplies
    layer-independent nbr/adj reductions (Gram term, nbrsum@W3, S1/S1a);
    device adds the fea-dependent linear/quadratic terms via tiny matmul
    contractions; per layer one 8-core AllGather of [128,6] partial sums
    (cheaper floor than AllReduce) + local 8-slot tree reduce.
  - sigmoid via ACT Sigmoid table (bn1 folded into per-partition scale/bias);
    core-half drained from PSUM on DVE with the bn1 affine folded in; softplus
    per chunk as Exp+Ln on ACT (natural_log_exp table, 2 table loads/layer).
  - h = sig*sp on DVE; k-sum via contiguous-halves add tree (bf16 2x).
  - bn2: free-dim reduce + AllGather [128,2] + local reduce; fea update
    via Softplus.
"""

import numpy as np
import ml_dtypes

import concourse.bass as bass
import concourse.mybir as mybir
from concourse import tile

F32 = mybir.dt.float32
BF16 = mybir.dt.bfloat16
I32 = mybir.dt.int32
AF = mybir.ActivationFunctionType
OP = mybir.AluOpType

EPS = 1e-5
N0, N, ORIG, F, K, H, NC = 16, 96, 92, 64, 41, 128, 3
NCORES, BPC = 8, 2
BJ = BPC * N            # 192
R = BJ * N              # 18432
G32 = 32
NBLK = BJ // G32        # 6
HALF = R // 2           # 9216
NTOT = float(N0 * N * N)
NTOT2 = float(N0 * N)
NGRP = 18               # main groups per layer, 512 paired-cols each
GW = 512
SPCH = 3                # softplus/mul/tree chunks
CHW = HALF // SPCH      # 2304 = 24 bj * 96


def bf16(x):
    return np.ascontiguousarray(np.asarray(x, np.float32).astype(ml_dtypes.bfloat16))


INPUT_SPECS = [
    ("rhs_pack", (128, R), BF16),
    ("atomT", (ORIG + 1, BJ), F32),
    ("emb", (ORIG + 1, F), F32),
    ("w3", (K, NC * 128), BF16),
    ("wab", (F + 1, NC * 256), BF16),
    ("s1s", (G32, NC * NBLK * 256), BF16),
    ("aux", (F, 8), BF16),
    ("gvec", (128, 12), F32),
    ("gvec2", (F, 6), F32),
    ("fcW", (F, H), F32),
    ("fcb", (H, 1), F32),
    ("outW", (H, 1), F32),
    ("outb", (1, 1), F32),
]


def host_prep(inputs):
    """Build the 8 per-core input maps from the full problem inputs."""
    atom_fea = np.asarray(inputs["atom_fea"], np.float32)
    nbr_fea = np.asarray(inputs["nbr_fea"], np.float32)
    adj = np.asarray(inputs["adj"])
    conv_W = np.asarray(inputs["conv_W"], np.float64)
    conv_b = np.asarray(inputs["conv_b"], np.float64)

    emb_ext = np.concatenate(
        [np.asarray(inputs["emb_W"], np.float32),
         np.asarray(inputs["emb_b"], np.float32)[None]], 0)
    w3_all = np.concatenate([bf16(conv_W[l, 2 * F:]) for l in range(NC)], 1)
    wab_all = bf16(np.concatenate(
        [np.concatenate(
            [np.concatenate([conv_W[l, :F], conv_b[l][None]], 0),
             np.concatenate([conv_W[l, F:2 * F], np.zeros((1, 2 * F))], 0)], 1)
         for l in range(NC)], 1))
    fcW = np.asarray(inputs["fc_W"], np.float32)
    # negated: consumed as the Sigmoid nbias inside _softplus (see kernel)
    fcb = -np.asarray(inputs["fc_b"], np.float32).reshape(H, 1)
    outW = np.asarray(inputs["out_W"], np.float32).reshape(H, 1)
    outb = np.asarray(inputs["out_b"], np.float32).reshape(1, 1)
    bn1_g = np.asarray(inputs["bn1_g"], np.float32)
    bn1_b = np.asarray(inputs["bn1_b"], np.float32)
    bn2_g = np.asarray(inputs["bn2_g"], np.float32)
    bn2_b = np.asarray(inputs["bn2_b"], np.float32)

    colbj = np.arange(R) // N
    gidx = colbj % G32

    per_core, nbrsum_g, gram_g = [], 0.0, 0.0
    for c in range(NCORES):
        sl = slice(c * BPC, (c + 1) * BPC)
        nbr = nbr_fea[sl].reshape(R, K).astype(np.float64)
        adjf = adj[sl].reshape(R).astype(np.float64)
        deg = adjf.reshape(BJ, N).sum(1)
        rhs = np.zeros((128, R), np.float32)
        rhs[0:K] = nbr.T
        rhs[64 + gidx, np.arange(R)] = 1.0
        rhs[96 + gidx, np.arange(R)] = adjf
        nbrj = nbr.reshape(BJ, N, K).sum(1)
        nbrja = (nbr.reshape(BJ, N, K) * adjf.reshape(BJ, N, 1)).sum(1)
        s1s = np.empty((G32, NC * NBLK * 256), np.float64)
        for l in range(NC):
            W3 = conv_W[l, 2 * F:]
            S1T, S1aT = nbrj @ W3, nbrja @ W3
            for b in range(NBLK):
                blk = np.concatenate(
                    [S1T[b * G32:(b + 1) * G32], S1aT[b * G32:(b + 1) * G32]], 1)
                s1s[:, (l * NBLK + b) * 256:(l * NBLK + b + 1) * 256] = blk
        aux = np.zeros((F, 8), np.float64)
        for b in range(NBLK):
            aux[0:32, b] = deg[b * G32:(b + 1) * G32]
            aux[32:64, b] = deg[b * G32:(b + 1) * G32]
        aux[0:64, 6] = 1.0
        atomT = np.concatenate(
            [atom_fea[sl].reshape(BJ, ORIG).T, np.ones((1, BJ))], 0).astype(np.float32)
        nbrsum_g = nbrsum_g + nbr.sum(0)
        gram_g = gram_g + nbr.T @ nbr
        per_core.append(dict(rhs=bf16(rhs), atomT=atomT, s1s=bf16(s1s), aux=bf16(aux)))

    # Core-half bn1 params and bn2 gain are negated host-side: the kernel
    # computes softplus(z) as -Ln(Sigmoid(-z)), so the core affine must
    # produce -z, and the resulting negated `summed` is fixed up in bn2 by
    # the negated gain (bias formula is sign-invariant).
    gvec = np.zeros((128, 12), np.float32)
    for l in range(NC):
        W3 = conv_W[l, 2 * F:]
        gvec[:, l] = nbrsum_g @ W3
        gvec[:, 3 + l] = np.einsum("fc,fg,gc->c", W3, gram_g, W3)
        gvec[:, 6 + l] = bn1_g[l]
        gvec[F:128, 6 + l] *= -1.0
        gvec[:, 9 + l] = bn1_b[l]
        gvec[F:128, 9 + l] *= -1.0
    gvec2 = np.zeros((F, 6), np.float32)
    for l in range(NC):
        gvec2[:, l] = -bn2_g[l]
        gvec2[:, 3 + l] = bn2_b[l]

    in_maps = []
    for c in range(NCORES):
        pc = per_core[c]
        in_maps.append({
            "rhs_pack": pc["rhs"], "atomT": pc["atomT"], "emb": emb_ext,
            "w3": w3_all, "wab": wab_all, "s1s": pc["s1s"], "aux": pc["aux"],
            "gvec": gvec, "gvec2": gvec2, "fcW": fcW, "fcb": fcb,
            "outW": outW, "outb": outb,
        })
    return in_maps


def _softplus(nc, pool, out, in_, tag, nbias=0.0, nscale=-1.0):
    """out = softplus(x) via -Ln(Sigmoid(-x)); pass nscale=-scale, nbias=-bias.

    Using only {Sigmoid, Ln} keeps every ACT in the kernel inside two
    tables (sigmoid_and_others / natural_log) instead of thrashing the
    Exp<->Ln table pair on every softplus."""
    p, fd = out.shape[0], int(np.prod(out.shape[1:]))
    e = pool.tile([p, fd], F32, tag=tag + "_e")
    nc.scalar.activation(e[:], in_, AF.Sigmoid, bias=nbias, scale=nscale)
    nc.scalar.activation(out, e[:], AF.Ln)
    nc.vector.tensor_scalar(out, out, -1.0, None, OP.mult)


def _rsqrt(nc, pool, out, v, p):
    """out = 1/sqrt(v), [p,1] f32, via magic-init + 3 Newton iterations."""
    yb = pool.tile([p, 1], I32, tag="rs_i")
    nc.vector.tensor_scalar(yb[:], v.bitcast(I32), 1, None, OP.logical_shift_right)
    nc.vector.tensor_scalar(yb[:], yb[:], -1, 0x5F3759DF, OP.mult, OP.add)
    y = yb.bitcast(F32)
    t = pool.tile([p, 1], F32, tag="rs_t")
    for _ in range(2):
        nc.vector.tensor_tensor(t[:], y[:], y[:], OP.mult)
        nc.vector.tensor_tensor(t[:], t[:], v[:], OP.mult)
        nc.vector.tensor_scalar(t[:], t[:], -0.5, 1.5, OP.mult, OP.add)
        nc.vector.tensor_tensor(y[:], y[:], t[:], OP.mult)
    nc.vector.tensor_copy(out, y[:])


def trace_body(nc, d, out_ap):
    """d: dict name -> DRAM AP (inputs); out_ap: [1,2] f32 DRAM output."""
    # Collective bounce buffers: outputs must be addr_space="Shared" on HW.
    # AllGather (floor ~2x cheaper than AllReduce) + local 8-slot reduce.
    ar1b = [(nc.dram_tensor(f"ar1i_{l}", [128, 6], F32).ap(),
             nc.dram_tensor(f"ar1o_{l}", [NCORES, 128, 6], F32,
                            addr_space="Shared").ap())
            for l in range(NC)]
    ar2b = [(nc.dram_tensor(f"ar2i_{l}", [128, 2], F32).ap(),
             nc.dram_tensor(f"ar2o_{l}", [NCORES, 128, 2], F32,
                            addr_space="Shared").ap())
            for l in range(NC)]
    with tile.TileContext(nc) as tc:
        with (
            tc.tile_pool(name="big", bufs=1) as big,
            tc.tile_pool(name="cst", bufs=1) as cst,
            tc.tile_pool(name="ph0", bufs=2) as ph0,
            tc.tile_pool(name="gate", bufs=2) as gate,
            tc.tile_pool(name="sm", bufs=2) as sm,
            tc.tile_pool(name="ps_ab", bufs=1, space="PSUM") as ps_ab_p,
            tc.tile_pool(name="ps_st", bufs=1, space="PSUM") as ps_st_p,
            tc.tile_pool(name="ps_f", bufs=3, space="PSUM") as ps_f_p,
            tc.tile_pool(name="ps_c", bufs=2, space="PSUM") as ps_c_p,
            tc.tile_pool(name="ps_m", bufs=1, space="PSUM") as ps_m_p,
        ):
            # ---- load constants (rhs_pack last: only needed by the main
            # matmuls ~40us in; issuing it first would stall the small loads
            # behind a 14us DMA and delay phase0 + the first AllGather) ----
            rhs_pack = big.tile([128, R], BF16)
            atomT = cst.tile([ORIG + 1, BJ], F32)
            nc.sync.dma_start(atomT[:], d["atomT"])
            emb = cst.tile([ORIG + 1, F], F32)
            nc.sync.dma_start(emb[:], d["emb"])
            w3 = cst.tile([K, NC * 128], BF16)
            nc.sync.dma_start(w3[:], d["w3"])
            wab = cst.tile([F + 1, NC * 256], BF16)
            nc.sync.dma_start(wab[:], d["wab"])
            s1s = cst.tile([G32, NC * NBLK * 256], BF16)
            nc.sync.dma_start(s1s[:], d["s1s"])
            aux = cst.tile([F, 8], BF16)
            nc.sync.dma_start(aux[:], d["aux"])
            gvec = cst.tile([128, 12], F32)
            nc.sync.dma_start(gvec[:], d["gvec"])
            gvec2 = cst.tile([F, 6], F32)
            nc.sync.dma_start(gvec2[:], d["gvec2"])
            fcW = cst.tile([F, H], F32)
            nc.sync.dma_start(fcW[:], d["fcW"])
            fcb = cst.tile([H, 1], F32)
            nc.sync.dma_start(fcb[:], d["fcb"])
            outW = cst.tile([H, 1], F32)
            nc.sync.dma_start(outW[:], d["outW"])
            outb = cst.tile([1, 1], F32)
            nc.sync.dma_start(outb[:], d["outb"])
            nc.sync.dma_start(rhs_pack[:], d["rhs_pack"])

            # ---- embedding: fea_ext [65, 192] = [(atom@embW+b)^T ; ones] ----
            ps_e = ps_m_p.tile([F, BJ], F32, tag="pm")
            nc.tensor.matmul(ps_e[:], emb[:], atomT[:], start=True, stop=True)
            fea_ext = ph0.tile([F + 1, BJ], F32, tag="fea")
            nc.vector.tensor_copy(fea_ext[0:F, :], ps_e[:])
            nc.vector.memset(fea_ext[F:F + 1, :], 1.0)
            # bf16 shadow of fea_ext: keeps the wab matmuls in bf16 mode
            # (fp32 matmul streams at 1/4 rate, 2x LDWEIGHTS)
            feaB = ph0.tile([F + 1, BJ], BF16, tag="feaB")
            nc.vector.tensor_copy(feaB[0:F, :], ps_e[:])
            nc.vector.memset(feaB[F:F + 1, :], 1.0)

            for l in range(NC):
                # ================= phase 0: A'/B, lhsT blocks, bn1 stat terms
                lhs_all = ph0.tile([128, NBLK * 2 * F], BF16, tag="lhs")
                nc.vector.memset(lhs_all[:], 0.0)  # rows 41:64 must stay zero
                ps_st = ps_st_p.tile([128, 8], F32, tag="st")
                # ab2 [32, 256] per block = [A'^T | B^T] at base partition 0
                # (TT inputs must share base partition when both in SBUF);
                # kept for all 6 blocks so lhsT assembly can run AFTER the
                # AR1 trigger, inside the collective's latency shadow.
                ab2_all = ph0.tile([G32, NBLK * 256], BF16, tag="ab_sb")
                for b in range(NBLK):
                    o = b * 256
                    ps_ab = ps_ab_p.tile([128, 128], F32, tag="ab")
                    fsl = feaB[:, b * G32:(b + 1) * G32]
                    nc.tensor.matmul(ps_ab[64:96, :], fsl,
                                     wab[:, l * 256:l * 256 + 128],
                                     start=True, stop=True, tile_position=(0, 64))
                    nc.tensor.matmul(ps_ab[96:128, :], fsl,
                                     wab[:, l * 256 + 128:l * 256 + 256],
                                     start=True, stop=True, tile_position=(0, 96))
                    nc.vector.tensor_copy(ab2_all[:, o:o + 128], ps_ab[64:96, :])
                    nc.vector.tensor_copy(ab2_all[:, o + 128:o + 256],
                                          ps_ab[96:128, :])
                    sq = ph0.tile([G32, 256], BF16, tag="sq")
                    nc.vector.tensor_tensor(sq[:], ab2_all[:, o:o + 256],
                                            ab2_all[:, o:o + 256], OP.mult)
                    prod = ph0.tile([G32, 128], BF16, tag="prod")
                    nc.vector.tensor_tensor(prod[:], ab2_all[:, o:o + 128],
                                            ab2_all[:, o + 128:o + 256], OP.mult)
                    crs = ph0.tile([G32, 256], BF16, tag="crs")
                    nc.vector.tensor_tensor(
                        crs[:], ab2_all[:, o:o + 256],
                        s1s[:, (l * NBLK + b) * 256:(l * NBLK + b + 1) * 256],
                        OP.mult)
                    # stat contractions -> ps_st cols 0..5
                    st, sp_ = (b == 0), (b == NBLK - 1)
                    nc.tensor.matmul(ps_st[:, 0:1], ab2_all[:, o:o + 128],
                                     aux[0:32, 6:7],
                                     start=st, stop=sp_, skip_group_check=True)
                    nc.tensor.matmul(ps_st[:, 1:2], ab2_all[:, o + 128:o + 256],
                                     aux[0:32, b:b + 1],
                                     start=st, stop=sp_, skip_group_check=True)
                    nc.tensor.matmul(ps_st[:, 2:3], sq[:, 0:128], aux[0:32, 6:7],
                                     start=st, stop=sp_, skip_group_check=True)
                    nc.tensor.matmul(ps_st[:, 3:4], sq[:, 128:256],
                                     aux[0:32, b:b + 1],
                                     start=st, stop=sp_, skip_group_check=True)
                    nc.tensor.matmul(ps_st[:, 4:5], prod[:], aux[0:32, b:b + 1],
                                     start=st, stop=sp_, skip_group_check=True)
                    nc.tensor.matmul(ps_st[:, 5:6], crs[:, 0:128], aux[0:32, 6:7],
                                     start=st, stop=False, skip_group_check=True)
                    nc.tensor.matmul(ps_st[:, 5:6], crs[:, 128:256], aux[0:32, 6:7],
                                     start=False, stop=sp_, skip_group_check=True)

                # ---- AR1: AllGather partials + local 8-slot tree reduce ----
                ar_sb = sm.tile([128, 6], F32, tag="ar1s")
                nc.vector.tensor_copy(ar_sb[:], ps_st[:, 0:6])
                ar_in, ar_out = ar1b[l]
                nc.sync.dma_start(ar_in, ar_sb[:])
                nc.gpsimd.collective_compute(
                    "AllGather", OP.bypass, replica_groups=[list(range(NCORES))],
                    ins=[ar_in], outs=[ar_out])

                # lhsT assembly (W3 + A'/B rows), hidden under AR1 latency
                for b in range(NBLK):
                    o = b * 256
                    fcol, ccol = 2 * b * F, (2 * b + 1) * F
                    nc.vector.tensor_copy(lhs_all[0:K, fcol:fcol + F],
                                          w3[:, l * 128:l * 128 + F])
                    nc.vector.tensor_copy(lhs_all[0:K, ccol:ccol + F],
                                          w3[:, l * 128 + F:(l + 1) * 128])
                    nc.vector.tensor_copy(lhs_all[64:96, fcol:fcol + F],
                                          ab2_all[:, o:o + F])
                    nc.vector.tensor_copy(lhs_all[64:96, ccol:ccol + F],
                                          ab2_all[:, o + F:o + 128])
                    nc.vector.tensor_copy(lhs_all[96:128, fcol:fcol + F],
                                          ab2_all[:, o + 128:o + 128 + F])
                    nc.vector.tensor_copy(lhs_all[96:128, ccol:ccol + F],
                                          ab2_all[:, o + 128 + F:o + 256])

                argg = sm.tile([128, NCORES * 6], F32, tag="ar1gg")
                nc.sync.dma_start(
                    argg[:], ar_out.rearrange("r p c -> p r c"))
                arg4 = sm.tile([128, 24], F32, tag="ar1g4")
                nc.vector.tensor_tensor(arg4[:], argg[:, 0:24], argg[:, 24:48],
                                        OP.add)
                arg2r = sm.tile([128, 12], F32, tag="ar1g2")
                nc.vector.tensor_tensor(arg2r[:], arg4[:, 0:12], arg4[:, 12:24],
                                        OP.add)
                arg = sm.tile([128, 6], F32, tag="ar1g")
                nc.vector.tensor_tensor(arg[:], arg2r[:, 0:6], arg2r[:, 6:12],
                                        OP.add)

                # ---- main matmuls (raw gated) + sig + drain ----
                sig_buf = gate.tile([128, HALF], BF16, tag="sig", bufs=1)
                spin_buf = gate.tile([128, HALF], BF16, tag="spin", bufs=1)

                # bn1 finalize (tiny, f32)  -- runs when arg ready
                sg = sm.tile([128, 1], F32, tag="sg")
                nc.vector.tensor_scalar(sg[:], arg[:, 0:1], float(N), None, OP.mult)
                nc.vector.tensor_tensor(sg[:], sg[:], arg[:, 1:2], OP.add)
                nc.vector.tensor_tensor(sg[:], sg[:], gvec[:, l:l + 1], OP.add)
                sg2 = sm.tile([128, 1], F32, tag="sg2")
                nc.vector.tensor_scalar(sg2[:], arg[:, 2:3], float(N), None, OP.mult)
                nc.vector.tensor_tensor(sg2[:], sg2[:], arg[:, 3:4], OP.add)
                t45 = sm.tile([128, 1], F32, tag="t45")
                nc.vector.tensor_tensor(t45[:], arg[:, 4:5], arg[:, 5:6], OP.add)
                nc.vector.tensor_scalar(t45[:], t45[:], 2.0, None, OP.mult)
                nc.vector.tensor_tensor(sg2[:], sg2[:], t45[:], OP.add)
                nc.vector.tensor_tensor(sg2[:], sg2[:], gvec[:, 3 + l:4 + l], OP.add)
                mean = sm.tile([128, 1], F32, tag="mean")
                nc.vector.tensor_scalar(mean[:], sg[:], 1.0 / NTOT, None, OP.mult)
                var = sm.tile([128, 1], F32, tag="var")
                nc.vector.tensor_tensor(var[:], mean[:], mean[:], OP.mult)
                ex2 = sm.tile([128, 1], F32, tag="ex2")
                nc.vector.tensor_scalar(ex2[:], sg2[:], 1.0 / NTOT, None, OP.mult)
                nc.vector.tensor_tensor(var[:], ex2[:], var[:], OP.subtract)
                nc.vector.tensor_scalar(var[:], var[:], EPS, None, OP.add)
                inv = sm.tile([128, 1], F32, tag="inv")
                _rsqrt(nc, sm, inv[:], var, 128)
                scl = sm.tile([128, 1], F32, tag="scl")
                nc.vector.tensor_tensor(scl[:], gvec[:, 6 + l:7 + l], inv[:], OP.mult)
                bia = sm.tile([128, 1], F32, tag="bia")
                nc.vector.tensor_tensor(bia[:], mean[:], scl[:], OP.mult)
                nc.vector.tensor_tensor(bia[:], gvec[:, 9 + l:10 + l], bia[:],
                                        OP.subtract)
                # filt-half bn1 affine, duplicated to both partition halves
                sigscl = sm.tile([128, 1], F32, tag="sigscl")
                nc.vector.tensor_copy(sigscl[0:F, :], scl[0:F, :])
                nc.vector.tensor_copy(sigscl[F:128, :], scl[0:F, :])
                sigbia = sm.tile([128, 1], F32, tag="sigbia")
                nc.vector.tensor_copy(sigbia[0:F, :], bia[0:F, :])
                nc.vector.tensor_copy(sigbia[F:128, :], bia[0:F, :])
                spscl = sm.tile([128, 1], F32, tag="spscl")
                nc.vector.tensor_copy(spscl[0:F, :], scl[F:128, :])
                nc.vector.tensor_copy(spscl[F:128, :], scl[F:128, :])
                spbia = sm.tile([128, 1], F32, tag="spbia")
                nc.vector.tensor_copy(spbia[0:F, :], bia[F:128, :])
                nc.vector.tensor_copy(spbia[F:128, :], bia[F:128, :])

                for g in range(NGRP):
                    bp, j = g // 6, g % 6
                    c0 = bp * 3072 + j * GW
                    c1 = HALF + c0
                    gc = c0
                    ps_f = ps_f_p.tile([128, GW], F32, tag="psf")
                    ps_c = ps_c_p.tile([128, GW], F32, tag="psc")
                    fa = lhs_all[:, 2 * bp * F:(2 * bp + 1) * F]
                    fb = lhs_all[:, 2 * (bp + 3) * F:(2 * (bp + 3) + 1) * F]
                    ca = lhs_all[:, (2 * bp + 1) * F:(2 * bp + 2) * F]
                    cb = lhs_all[:, (2 * (bp + 3) + 1) * F:(2 * (bp + 3) + 2) * F]
                    nc.tensor.matmul(ps_f[0:F, :], fa, rhs_pack[:, c0:c0 + GW],
                                     start=True, stop=True)
                    nc.tensor.matmul(ps_f[F:128, :], fb, rhs_pack[:, c1:c1 + GW],
                                     start=True, stop=True)
                    nc.tensor.matmul(ps_c[0:F, :], ca, rhs_pack[:, c0:c0 + GW],
                                     start=True, stop=True)
                    nc.tensor.matmul(ps_c[F:128, :], cb, rhs_pack[:, c1:c1 + GW],
                                     start=True, stop=True)
                    nc.scalar.activation(sig_buf[:, gc:gc + GW], ps_f[:],
                                         AF.Sigmoid, bias=sigbia[:],
                                         scale=sigscl[:])
                    nc.vector.tensor_scalar(spin_buf[:, gc:gc + GW], ps_c[:],
                                            spscl[:], spbia[:],
                                            OP.mult, OP.add)

                # ---- softplus + mul + k-reduce tree, chunked pipeline ----
                summed = sm.tile([128, 2 * BPC * G32 * 3 // 4], F32, tag="summed")
                # summed [128, 96]: p<64 -> (c, bj 0:96), p>=64 -> (c, bj 96:192)
                # spin holds -(bn1 core affine); Sigmoid+Ln gives -softplus,
                # absorbed by the host-negated bn2 gain. All Sigmoids are
                # emitted before any Ln so the ACT table switches once.
                s_all = gate.tile([128, HALF], BF16, tag="sall", bufs=1)
                for ch in range(SPCH):
                    co = ch * CHW
                    nc.scalar.activation(s_all[:, co:co + CHW],
                                         spin_buf[:, co:co + CHW], AF.Sigmoid)
                for ch in range(SPCH):
                    co = ch * CHW
                    nbj = CHW // N  # 24
                    sp_t = gate.tile([128, CHW], BF16, tag="sp")
                    nc.scalar.activation(sp_t[:], s_all[:, co:co + CHW], AF.Ln)
                    h_t = gate.tile([128, CHW], BF16, tag="h")
                    nc.vector.tensor_tensor(h_t[:], sig_buf[:, co:co + CHW],
                                            sp_t[:], OP.mult)
                    # tree: 96 -> 48 -> 24 -> 12 -> 6 -> 3 -> (2 adds)
                    w = N
                    cur = h_t
                    while w > 3:
                        nw = w // 2
                        nxt = gate.tile([128, nbj * nw], BF16, tag=f"tr{w}")
                        va = cur[:].rearrange("p (b k) -> p b k", k=w)
                        nc.vector.tensor_tensor(
                            nxt[:].rearrange("p (b k) -> p b k", k=nw),
                            va[:, :, 0:nw], va[:, :, nw:2 * nw], OP.add)
                        cur, w = nxt, nw
                    va = cur[:].rearrange("p (b k) -> p b k", k=3)
                    s01 = gate.tile([128, nbj], F32, tag="s01")
                    nc.vector.tensor_tensor(
                        s01[:].rearrange("p (b k) -> p b k", k=1),
                        va[:, :, 0:1], va[:, :, 1:2], OP.add)
                    nc.vector.tensor_tensor(
                        summed[:, ch * nbj:(ch + 1) * nbj].rearrange(
                            "p (b k) -> p b k", k=1),
                        s01[:].rearrange("p (b k) -> p b k", k=1),
                        va[:, :, 2:3], OP.add)

                # ---- bn2 ----
                NB2 = 2 * BPC * G32 * 3 // 4  # 96
                ar2_sb = sm.tile([128, 2], F32, tag="ar2s")
                nc.vector.tensor_reduce(ar2_sb[:, 0:1], summed[:],
                                        axis=mybir.AxisListType.X, op=OP.add)
                ssq = sm.tile([128, NB2], F32, tag="ssq")
                nc.vector.tensor_tensor(ssq[:], summed[:], summed[:], OP.mult)
                nc.vector.tensor_reduce(ar2_sb[:, 1:2], ssq[:],
                                        axis=mybir.AxisListType.X, op=OP.add)
                ar2_in, ar2_out = ar2b[l]
                nc.sync.dma_start(ar2_in, ar2_sb[:])
                nc.gpsimd.collective_compute(
                    "AllGather", OP.bypass, replica_groups=[list(range(NCORES))],
                    ins=[ar2_in], outs=[ar2_out])
                ar2gg = sm.tile([128, NCORES * 2], F32, tag="ar2gg")
                nc.sync.dma_start(
                    ar2gg[:], ar2_out.rearrange("r p c -> p r c"))
                ar2g4 = sm.tile([128, 8], F32, tag="ar2g4")
                nc.vector.tensor_tensor(ar2g4[:], ar2gg[:, 0:8], ar2gg[:, 8:16],
                                        OP.add)
                ar2g2 = sm.tile([128, 4], F32, tag="ar2g2")
                nc.vector.tensor_tensor(ar2g2[:], ar2g4[:, 0:4], ar2g4[:, 4:8],
                                        OP.add)
                arg2 = sm.tile([128, 2], F32, tag="ar2g")
                nc.vector.tensor_tensor(arg2[:], ar2g2[:, 0:2], ar2g2[:, 2:4],
                                        OP.add)
                arg2c = sm.tile([F, 2], F32, tag="arg2c")
                nc.vector.tensor_copy(arg2c[:], arg2[F:128, :])
                g2 = sm.tile([F, 2], F32, tag="g2")
                nc.vector.tensor_tensor(g2[:], arg2[0:F, :], arg2c[:], OP.add)
                m2 = sm.tile([F, 1], F32, tag="m2")
                nc.vector.tensor_scalar(m2[:], g2[:, 0:1], 1.0 / NTOT2, None, OP.mult)
                v2 = sm.tile([F, 1], F32, tag="v2")
                nc.vector.tensor_tensor(v2[:], m2[:], m2[:], OP.mult)
                e2 = sm.tile([F, 1], F32, tag="e2")
                nc.vector.tensor_scalar(e2[:], g2[:, 1:2], 1.0 / NTOT2, None, OP.mult)
                nc.vector.tensor_tensor(v2[:], e2[:], v2[:], OP.subtract)
                nc.vector.tensor_scalar(v2[:], v2[:], EPS, None, OP.add)
                i2 = sm.tile([F, 1], F32, tag="i2")
                _rsqrt(nc, sm, i2[:], v2, F)
                s2 = sm.tile([F, 1], F32, tag="s2")
                nc.vector.tensor_tensor(s2[:], gvec2[:, l:l + 1], i2[:], OP.mult)
                b2 = sm.tile([F, 1], F32, tag="b2")
                nc.vector.tensor_tensor(b2[:], m2[:], s2[:], OP.mult)
                nc.vector.tensor_tensor(b2[:], gvec2[:, 3 + l:4 + l], b2[:],
                                        OP.subtract)
                s2d = sm.tile([128, 1], F32, tag="s2d")
                nc.vector.tensor_copy(s2d[0:F, :], s2[:])
                nc.vector.tensor_copy(s2d[F:128, :], s2[:])
                b2d = sm.tile([128, 1], F32, tag="b2d")
                nc.vector.tensor_copy(b2d[0:F, :], b2[:])
                nc.vector.tensor_copy(b2d[F:128, :], b2[:])
                sn = sm.tile([128, NB2], F32, tag="sn")
                nc.vector.tensor_scalar(sn[:], summed[:], s2d[:], b2d[:],
                                        OP.mult, OP.add)
                snc = sm.tile([F, NB2], F32, tag="snc")
                nc.vector.tensor_copy(snc[:], sn[F:128, :])
                tmp = sm.tile([F, BJ], F32, tag="tmpf")
                nc.vector.tensor_tensor(tmp[:, 0:N], sn[0:F, :],
                                        fea_ext[0:F, 0:N], OP.add)
                nc.vector.tensor_tensor(tmp[:, N:BJ], snc[:],
                                        fea_ext[0:F, N:BJ], OP.add)
                fea_new = ph0.tile([F + 1, BJ], F32, tag="fea")
                _softplus(nc, sm, fea_new[0:F, :], tmp[:], "feasp")
                nc.vector.memset(fea_new[F:F + 1, :], 1.0)
                fea_ext = fea_new
                feaB = ph0.tile([F + 1, BJ], BF16, tag="feaB")
                nc.vector.tensor_copy(feaB[0:F, :], fea_new[0:F, :])
                nc.vector.memset(feaB[F:F + 1, :], 1.0)

            # ---- head ----
            crys = sm.tile([F, BPC], F32, tag="crys")
            nc.vector.tensor_reduce(
                crys[:], fea_ext[0:F, :].rearrange("p (a b) -> p a b", b=N),
                axis=mybir.AxisListType.X, op=OP.add)
            nc.vector.tensor_scalar(crys[:], crys[:], 1.0 / N, None, OP.mult)
            crys2 = sm.tile([F, BPC], F32, tag="crys2")
            _softplus(nc, sm, crys2[:], crys[:], "hd1")
            ps_h = ps_m_p.tile([H, BPC], F32, tag="pm")
            nc.tensor.matmul(ps_h[:], fcW[:], crys2[:], start=True, stop=True)
            sph = sm.tile([H, BPC], F32, tag="sph")
            _softplus(nc, sm, sph[:], ps_h[:], "hd2", nbias=fcb[:])
            ps_o = ps_m_p.tile([1, BPC], F32, tag="pm")
            nc.tensor.matmul(ps_o[:], outW[:], sph[:], start=True, stop=True)
            res = sm.tile([1, BPC], F32, tag="res")
            nc.vector.tensor_scalar(res[:], ps_o[:], outb[0:1, 0:1], None, OP.add)
            nc.sync.dma_start(out_ap, res[:])  # out dram is [1, BPC]
    return nc


# ======================================================================
# Self-contained runner: shard -> compile (cached) -> run SPMD -> gather
# ======================================================================
_COMPILED = {}


def _build_nc():
    import concourse.bacc as bacc
    nc = bacc.Bacc("TRN2", target_bir_lowering=False, debug=False,
                   num_devices=NCORES)
    d = {}
    for name, shape, dt in INPUT_SPECS:
        d[name] = nc.dram_tensor(name, list(shape), dt, kind="ExternalInput").ap()
    out_ap = nc.dram_tensor("out", [1, BPC], F32, kind="ExternalOutput").ap()
    trace_body(nc, d, out_ap)
    nc.compile()
    return nc


def kernel(**inputs):
    from concourse.bass_utils import run_bass_kernel_spmd
    in_maps = host_prep(inputs)
    if "nc" not in _COMPILED:
        _COMPILED["nc"] = _build_nc()
    nc = _COMPILED["nc"]
    res = run_bass_kernel_spmd(nc, in_maps, core_ids=list(range(NCORES)))
    out = np.concatenate([np.asarray(r["out"], np.float32).reshape(BPC)
                          for r in res.results])
    return out.reshape(N0, 1)

